# revision 9
# baseline (speedup 1.0000x reference)
"""Trainium2 Bass kernel for nn_PointOutlierPooling.

Strategy (8 cores, data-parallel over batch B=16 -> 2 rows/core):
  - fp32 main MLP feature-major (weights stationary as lhsT), Lrelu/Relu on ACT
  - probs = (h3 @ Wp + bp) / ||Wp||  -> full descending argsort per row on device
    via a uniform XOR-partner bitonic network (free-dim partners via strided
    copies, cross-partition partners via PE permutation matmuls), non-stable
    ties fixed to index-ascending by post-passes
  - displacement MLP in bf16 (d is a tiny correction to est_xyz)
  - est_full = xyz + d computed densely for all N; host gathers clean rows
"""
import os
import numpy as np
import concourse.bass as bass
import concourse.bacc as bacc
import concourse.tile as tile
from concourse import mybir
from concourse.bass_utils import run_bass_kernel_spmd

dt = mybir.dt
AF = mybir.ActivationFunctionType
OP = mybir.AluOpType

B_FULL = 16
N_FULL = 32768
PC_C, IN_C, H = 3, 128, 256
N_CORES = 8
NB = B_FULL // N_CORES          # batches per core
PERCENT = 0.1


def build_program(NB, N, c_norm, bp_val, enable_sort=True, sim_compat=False):
    """Build the per-core Bass program. c_norm = ||Wp||, bp_val = bp[0]."""
    C = N // 128                # free columns per partition in sort layout
    NCH = N // 512              # number of 512-pt chunks
    LMAX = int(np.log2(N))      # levels of the bitonic network
    assert 512 * NCH == N and 128 * C == N

    nc = bacc.Bacc(num_devices=N_CORES)
    fT = nc.declare_dram_parameter("fT", [NB, IN_C, N], dt.float32, isOutput=False)
    xyzT = nc.declare_dram_parameter("xyzT", [NB, PC_C, N], dt.float32, isOutput=False)
    W1 = nc.declare_dram_parameter("W1", [IN_C + PC_C, H], dt.float32, isOutput=False)
    b1 = nc.declare_dram_parameter("b1", [H], dt.float32, isOutput=False)
    W2 = nc.declare_dram_parameter("W2", [H, H], dt.float32, isOutput=False)
    b2 = nc.declare_dram_parameter("b2", [H], dt.float32, isOutput=False)
    W3 = nc.declare_dram_parameter("W3", [H, 32], dt.float32, isOutput=False)
    b3 = nc.declare_dram_parameter("b3", [32], dt.float32, isOutput=False)
    Wp = nc.declare_dram_parameter("Wp", [32, 1], dt.float32, isOutput=False)
    Wd1 = nc.declare_dram_parameter("Wd1", [IN_C, 64], dt.float32, isOutput=False)
    bd1 = nc.declare_dram_parameter("bd1", [64], dt.float32, isOutput=False)
    Wd2 = nc.declare_dram_parameter("Wd2", [64, 32], dt.float32, isOutput=False)
    bd2 = nc.declare_dram_parameter("bd2", [32], dt.float32, isOutput=False)
    Wd3 = nc.declare_dram_parameter("Wd3", [32, 3], dt.float32, isOutput=False)
    bd3 = nc.declare_dram_parameter("bd3", [3], dt.float32, isOutput=False)

    probs_o = nc.declare_dram_parameter("probs_o", [NB, N], dt.float32, isOutput=True)
    est_o = nc.declare_dram_parameter("est_o", [NB, PC_C, N], dt.float32, isOutput=True)
    pidx_o = nc.declare_dram_parameter("pidx_o", [NB, N], dt.int32, isOutput=True)

    inv_c = float(np.float32(1.0) / np.float32(c_norm))

    with tile.TileContext(nc) as tc:
        with (
            tc.tile_pool(name="wpool", bufs=1) as wpool,
            tc.tile_pool(name="iopool", bufs=3) as iopool,
            tc.tile_pool(name="hpool", bufs=2) as hpool,
            tc.tile_pool(name="sortpool", bufs=1) as sortpool,
            tc.tile_pool(name="mskpool", bufs=1) as mskpool,
            tc.tile_pool(name="ps1", bufs=2, space="PSUM") as ps1,
            tc.tile_pool(name="ps2", bufs=1, space="PSUM") as ps2,
            tc.tile_pool(name="ps3", bufs=1, space="PSUM") as ps3,
            tc.tile_pool(name="ps4", bufs=1, space="PSUM") as ps4,
            tc.tile_pool(name="ps5", bufs=1, space="PSUM") as ps5,
            tc.tile_pool(name="pss", bufs=1, space="PSUM") as pss,
        ):
            # ---------- weights / constants ----------
            W1t = {}
            for m in range(2):
                t = wpool.tile([128, 128], dt.float32, tag=f"W1a{m}")
                nc.sync.dma_start(t[:], W1[0:128, m * 128:(m + 1) * 128])
                W1t[(0, m)] = t
                t2 = wpool.tile([3, 128], dt.float32, tag=f"W1b{m}")
                nc.sync.dma_start(t2[:], W1[128:131, m * 128:(m + 1) * 128])
                W1t[(1, m)] = t2
            W2t = {}
            for k in range(2):
                for m in range(2):
                    t = wpool.tile([128, 128], dt.float32, tag=f"W2_{k}{m}")
                    nc.sync.dma_start(t[:], W2[k * 128:(k + 1) * 128, m * 128:(m + 1) * 128])
                    W2t[(k, m)] = t
            W3t = {}
            for k in range(2):
                t = wpool.tile([128, 32], dt.float32, tag=f"W3_{k}")
                nc.sync.dma_start(t[:], W3[k * 128:(k + 1) * 128, :])
                W3t[k] = t
            Wpt = wpool.tile([32, 1], dt.float32, tag="Wpt")
            nc.sync.dma_start(Wpt[:], Wp[:])

            def bias_tile(name, src, lo, n):
                t = wpool.tile([n, 1], dt.float32, tag=name)
                nc.sync.dma_start(t[:], src[:].rearrange("(p one) -> p one", one=1)[lo:lo + n])
                return t
            b1t = [bias_tile("b1t0", b1, 0, 128), bias_tile("b1t1", b1, 128, 128)]
            b2t = [bias_tile("b2t0", b2, 0, 128), bias_tile("b2t1", b2, 128, 128)]
            nb1t = nb2t = None
            if sim_compat:
                def neg_tile(name, src):
                    t = wpool.tile([128, 1], dt.float32, tag=name)
                    nc.scalar.activation(t[:], src[:], AF.Copy, scale=-1.0)
                    return t
                nb1t = [neg_tile("nb1t0", b1t[0]), neg_tile("nb1t1", b1t[1])]
                nb2t = [neg_tile("nb2t0", b2t[0]), neg_tile("nb2t1", b2t[1])]
            b3t = bias_tile("b3t", b3, 0, 32)
            bd1t = bias_tile("bd1t", bd1, 0, 64)
            bd2t = bias_tile("bd2t", bd2, 0, 32)
            bd3t = bias_tile("bd3t", bd3, 0, 3)

            # bf16 displacement weights
            Wd1f = wpool.tile([128, 64], dt.float32, tag="Wd1f")
            nc.sync.dma_start(Wd1f[:], Wd1[:])
            Wd1h = wpool.tile([128, 64], dt.bfloat16, tag="Wd1h")
            nc.vector.tensor_copy(Wd1h[:], Wd1f[:])
            Wd2f = wpool.tile([64, 32], dt.float32, tag="Wd2f")
            nc.sync.dma_start(Wd2f[:], Wd2[:])
            Wd2h = wpool.tile([64, 32], dt.bfloat16, tag="Wd2h")
            nc.vector.tensor_copy(Wd2h[:], Wd2f[:])
            Wd3f = wpool.tile([32, 3], dt.float32, tag="Wd3f")
            nc.sync.dma_start(Wd3f[:], Wd3[:])
            Wd3h = wpool.tile([32, 3], dt.bfloat16, tag="Wd3h")
            nc.vector.tensor_copy(Wd3h[:], Wd3f[:])

            ones16 = wpool.tile([1, 64], dt.bfloat16, tag="ones16")
            nc.vector.memset(ones16[:], 1.0)


            # sort constants: bit tiles over e = p*C + c, and partition-XOR perms
            bit_t = {}
            if enable_sort:
                e_i32 = wpool.tile([128, C], dt.int32, tag="e_i32")
                nc.gpsimd.iota(e_i32[:], pattern=[[1, C]], base=0, channel_multiplier=C)
                bscr = wpool.tile([128, C], dt.int32, tag="bscr")
                for b in range(LMAX):
                    bt = wpool.tile([128, C], dt.uint8, tag=f"bit{b}")
                    nc.vector.tensor_scalar(out=bscr[:], in0=e_i32[:], scalar1=b, scalar2=1,
                                            op0=OP.logical_shift_right, op1=OP.bitwise_and)
                    nc.vector.tensor_copy(bt[:], bscr[:])
                    bit_t[b] = bt
                ci = wpool.tile([128, 128], dt.int32, tag="perm_ci")
                nc.gpsimd.iota(ci[:], pattern=[[1, 128]], base=0, channel_multiplier=0)
                cif = wpool.tile([128, 128], dt.float32, tag="perm_cif")
                nc.vector.tensor_copy(cif[:], ci[:])
                perm = {}
                sp = 1
                while sp * C < N:
                    pi = wpool.tile([128, 1], dt.int32, tag=f"pi{sp}")
                    nc.gpsimd.iota(pi[:], pattern=[[0, 1]], base=0, channel_multiplier=1)
                    nc.vector.tensor_scalar(out=pi[:], in0=pi[:], scalar1=sp, scalar2=None,
                                            op0=OP.bitwise_xor)
                    pif = wpool.tile([128, 1], dt.float32, tag=f"pif{sp}")
                    nc.vector.tensor_copy(pif[:], pi[:])
                    pm = wpool.tile([128, 128], dt.float32, tag=f"perm{sp}")
                    nc.vector.scalar_tensor_tensor(out=pm[:], in0=cif[:], scalar=pif[:], in1=cif[:],
                                                   op0=OP.is_equal, op1=OP.bypass)
                    perm[sp] = pm
                    sp *= 2

            # ---------- per-batch state ----------
            KA = [sortpool.tile([128, C], dt.float32, tag=f"KA{b}", name=f"KA{b}") for b in range(NB)]
            KB = [sortpool.tile([128, C], dt.float32, tag=f"KB{b}", name=f"KB{b}") for b in range(NB)]
            IA = [sortpool.tile([128, C], dt.float32, tag=f"IA{b}", name=f"IA{b}") for b in range(NB)]
            IB = [sortpool.tile([128, C], dt.float32, tag=f"IB{b}", name=f"IB{b}") for b in range(NB)]

            # ---------- MLP over chunks ----------
            for b in range(NB):
                for ch in range(NCH):
                    c0 = ch * 512
                    fch = iopool.tile([128, 512], dt.float32, tag="fch")
                    nc.sync.dma_start(fch[:], fT[b, :, c0:c0 + 512])
                    xych = iopool.tile([3, 512], dt.float32, tag="xych")
                    nc.sync.dma_start(xych[:], xyzT[b, :, c0:c0 + 512])

                    h1 = []
                    for m in range(2):
                        z1 = ps1.tile([128, 512], dt.float32, tag="z1")
                        nc.tensor.matmul(z1[:], W1t[(0, m)][:], fch[:], start=True, stop=False)
                        nc.tensor.matmul(z1[:], W1t[(1, m)][:], xych[:], start=False, stop=True)
                        h = hpool.tile([128, 512], dt.float32, tag=f"h1_{m}")
                        if sim_compat:
                            hneg = hpool.tile([128, 512], dt.float32, tag=f"h1n_{m}")
                            nc.scalar.activation(h[:], z1[:], AF.Relu, bias=b1t[m][:])
                            nc.scalar.activation(hneg[:], z1[:], AF.Relu, bias=nb1t[m][:], scale=-1.0)
                            nc.vector.scalar_tensor_tensor(out=h[:], in0=hneg[:], scalar=-0.01,
                                                           in1=h[:], op0=OP.mult, op1=OP.add)
                        else:
                            nc.scalar.activation(h[:], z1[:], AF.Lrelu, bias=b1t[m][:], alpha=0.01)
                        h1.append(h)
                    h2 = []
                    for m in range(2):
                        z2 = ps1.tile([128, 512], dt.float32, tag="z2")
                        nc.tensor.matmul(z2[:], W2t[(0, m)][:], h1[0][:], start=True, stop=False)
                        nc.tensor.matmul(z2[:], W2t[(1, m)][:], h1[1][:], start=False, stop=True)
                        h = hpool.tile([128, 512], dt.float32, tag=f"h2_{m}")
                        if sim_compat:
                            hneg = hpool.tile([128, 512], dt.float32, tag=f"h2n_{m}")
                            nc.scalar.activation(h[:], z2[:], AF.Relu, bias=b2t[m][:])
                            nc.scalar.activation(hneg[:], z2[:], AF.Relu, bias=nb2t[m][:], scale=-1.0)
                            nc.vector.scalar_tensor_tensor(out=h[:], in0=hneg[:], scalar=-0.01,
                                                           in1=h[:], op0=OP.mult, op1=OP.add)
                        else:
                            nc.scalar.activation(h[:], z2[:], AF.Lrelu, bias=b2t[m][:], alpha=0.01)
                        h2.append(h)
                    z3zp = ps2.tile([33, 512], dt.float32, tag="z3zp")
                    nc.tensor.matmul(z3zp[0:32, :], W3t[0][:], h2[0][:], start=True, stop=False)
                    nc.tensor.matmul(z3zp[0:32, :], W3t[1][:], h2[1][:], start=False, stop=True)
                    h3 = hpool.tile([32, 512], dt.float32, tag="h3")
                    nc.scalar.activation(h3[:], z3zp[0:32, :], AF.Relu, bias=b3t[:])
                    nc.tensor.matmul(z3zp[32:33, :], Wpt[:], h3[:], start=True, stop=True,
                                     tile_position=(0, 32))
                    # probs_raw = zp + bp  (scale by 1/c exactly later)
                    pchunk = iopool.tile([1, 512], dt.float32, tag="pchunk")
                    nc.scalar.activation(pchunk[:], z3zp[32:33, :], AF.Copy, bias=float(bp_val))
                    nc.sync.dma_start(probs_o[b, c0:c0 + 512], pchunk[:])

                    # displacement path (bf16)
                    pb16 = iopool.tile([1, 512], dt.bfloat16, tag="pb16")
                    nc.vector.tensor_copy(pb16[:], pchunk[:])
                    fh16 = iopool.tile([128, 512], dt.bfloat16, tag="fh16")
                    nc.vector.tensor_copy(fh16[:], fch[:])
                    ybcu = ps3.tile([128, 512], dt.float32, tag="ybcu")
                    nc.tensor.matmul(ybcu[0:64, :], ones16[:], pb16[:], start=True, stop=True)
                    nc.tensor.matmul(ybcu[64:128, :], Wd1h[:], fh16[:], start=True, stop=True,
                                     tile_position=(0, 64))
                    y64 = iopool.tile([64, 512], dt.bfloat16, tag="y64")
                    nc.scalar.activation(y64[:], ybcu[0:64, :], AF.Sigmoid, scale=inv_c)
                    zd1 = iopool.tile([64, 512], dt.float32, tag="zd1")
                    nc.vector.tensor_tensor(out=zd1[:], in0=ybcu[64:128, :], in1=y64[:], op=OP.mult)
                    d1h = iopool.tile([64, 512], dt.bfloat16, tag="d1h")
                    nc.scalar.activation(d1h[:], zd1[:], AF.Relu, bias=bd1t[:])
                    d23 = ps4.tile([35, 512], dt.float32, tag="d23")
                    nc.tensor.matmul(d23[0:32, :], Wd2h[:], d1h[:], start=True, stop=True)
                    d2h = iopool.tile([32, 512], dt.bfloat16, tag="d2h")
                    nc.scalar.activation(d2h[:], d23[0:32, :], AF.Relu, bias=bd2t[:])
                    nc.tensor.matmul(d23[32:35, :], Wd3h[:], d2h[:], start=True, stop=True,
                                     tile_position=(0, 32))
                    est = iopool.tile([3, 512], dt.float32, tag="est")
                    nc.vector.scalar_tensor_tensor(out=est[:], in0=d23[32:35, :], scalar=bd3t[:],
                                                   in1=xych[:], op0=OP.add, op1=OP.add)
                    nc.sync.dma_start(est_o[b, :, c0:c0 + 512], est[:])

                # keys init for batch b (round-trip through probs_o DRAM)
                nc.sync.dma_start(KA[b][:], probs_o[b, :].rearrange("(p c) -> p c", p=128))
                nc.vector.tensor_scalar(out=KA[b][:], in0=KA[b][:], scalar1=inv_c,
                                        scalar2=None, op0=OP.mult)
                nc.sync.dma_start(probs_o[b, :], KA[b][:])
                idx_i = sortpool.tile([128, C], dt.int32, tag=f"idxi{b}")
                nc.gpsimd.iota(idx_i[:], pattern=[[1, C]], base=0, channel_multiplier=C)
                nc.vector.tensor_copy(IA[b][:], idx_i[:])

            # ---------- sort ----------
            for b in range(NB):
                if not enable_sort:
                    pid = sortpool.tile([128, C], dt.int32, tag=f"pid{b}")
                    nc.vector.tensor_copy(pid[:], IA[b][:])
                    nc.sync.dma_start(pidx_o[b, :], pid[:])
                    continue
                src_k, src_i, dst_k, dst_i = KA[b], IA[b], KB[b], IB[b]
                gt = mskpool.tile([128, C], dt.uint8, tag=f"gt{b}")
                eq = mskpool.tile([128, C], dt.uint8, tag=f"eq{b}")
                t1 = mskpool.tile([128, C], dt.uint8, tag=f"t1{b}")
                take = mskpool.tile([128, C], dt.uint8, tag=f"take{b}")
                for lev in range(1, LMAX + 1):
                    for j in range(lev - 1, -1, -1):
                        s = 1 << j
                        if s < C:
                            nb_ = C // (2 * s)
                            sv = src_k[:].rearrange("p (nb two s) -> p nb two s", two=2, s=s)
                            dv = dst_k[:].rearrange("p (nb two s) -> p nb two s", two=2, s=s)
                            nc.gpsimd.tensor_copy(dv[:, :, 0, :], sv[:, :, 1, :])
                            nc.gpsimd.tensor_copy(dv[:, :, 1, :], sv[:, :, 0, :])
                            svi = src_i[:].rearrange("p (nb two s) -> p nb two s", two=2, s=s)
                            dvi = dst_i[:].rearrange("p (nb two s) -> p nb two s", two=2, s=s)
                            nc.gpsimd.tensor_copy(dvi[:, :, 0, :], svi[:, :, 1, :])
                            nc.gpsimd.tensor_copy(dvi[:, :, 1, :], svi[:, :, 0, :])
                        else:
                            sp = s // C
                            bps = pss.tile([128, 2 * C], dt.float32, tag="bps")
                            nc.tensor.matmul(bps[:, 0:C], perm[sp][:], src_k[:], start=True, stop=True)
                            nc.tensor.matmul(bps[:, C:2 * C], perm[sp][:], src_i[:], start=True, stop=True)
                            nc.vector.tensor_copy(dst_k[:], bps[:, 0:C])
                            nc.vector.tensor_copy(dst_i[:], bps[:, C:2 * C])
                        nc.vector.tensor_tensor(out=gt[:], in0=src_k[:], in1=dst_k[:], op=OP.is_gt)
                        nc.vector.tensor_tensor(out=eq[:], in0=src_k[:], in1=dst_k[:], op=OP.is_equal)
                        nc.vector.tensor_tensor(out=t1[:], in0=gt[:], in1=bit_t[j][:], op=OP.logical_xor)
                        if lev < LMAX:
                            nc.vector.tensor_tensor(out=t1[:], in0=t1[:], in1=bit_t[lev][:], op=OP.logical_xor)
                        nc.vector.tensor_tensor(out=take[:], in0=t1[:], in1=eq[:], op=OP.logical_or)
                        nc.vector.copy_predicated(dst_k[:], take[:], src_k[:])
                        nc.vector.copy_predicated(dst_i[:], take[:], src_i[:])
                        src_k, dst_k = dst_k, src_k
                        src_i, dst_i = dst_i, src_i

                # ---- tie fix: equal keys -> ascending idx (stable) ----
                tmp = mskpool.tile([128, 128], dt.float32, tag=f"tfix{b}")
                for _ in range(3):
                    for q in (0, 1):
                        npair = (C - q) // 2
                        klo = src_k[:, q:q + 2 * npair].rearrange("p (t two) -> p t two", two=2)[:, :, 0]
                        khi = src_k[:, q:q + 2 * npair].rearrange("p (t two) -> p t two", two=2)[:, :, 1]
                        ilo = src_i[:, q:q + 2 * npair].rearrange("p (t two) -> p t two", two=2)[:, :, 0]
                        ihi = src_i[:, q:q + 2 * npair].rearrange("p (t two) -> p t two", two=2)[:, :, 1]
                        nc.vector.tensor_tensor(out=eq[:, 0:npair], in0=klo, in1=khi, op=OP.is_equal)
                        nc.vector.tensor_tensor(out=gt[:, 0:npair], in0=ilo, in1=ihi, op=OP.is_gt)
                        nc.vector.tensor_tensor(out=take[:, 0:npair], in0=eq[:, 0:npair],
                                                in1=gt[:, 0:npair], op=OP.logical_and)
                        nc.vector.tensor_copy(tmp[:, 0:npair], ihi)
                        nc.vector.copy_predicated(ihi, take[:, 0:npair], ilo)
                        nc.vector.copy_predicated(ilo, take[:, 0:npair], tmp[:, 0:npair])
                    # boundary pairs (p, C-1) <-> (p+1, 0)
                    bK = mskpool.tile([128, 2], dt.float32, tag=f"bK{b}")
                    bI = mskpool.tile([128, 2], dt.float32, tag=f"bI{b}")
                    nc.sync.dma_start(bK[0:127, 0:1], src_k[1:128, 0:1])
                    nc.sync.dma_start(bI[0:127, 0:1], src_i[1:128, 0:1])
                    nc.vector.tensor_tensor(out=eq[0:127, 0:1], in0=src_k[0:127, C - 1:C],
                                            in1=bK[0:127, 0:1], op=OP.is_equal)
                    nc.vector.tensor_tensor(out=gt[0:127, 0:1], in0=src_i[0:127, C - 1:C],
                                            in1=bI[0:127, 0:1], op=OP.is_gt)
                    nc.vector.tensor_tensor(out=take[0:127, 0:1], in0=eq[0:127, 0:1],
                                            in1=gt[0:127, 0:1], op=OP.logical_and)
                    nc.vector.tensor_copy(bI[0:127, 1:2], bI[0:127, 0:1])
                    nc.vector.copy_predicated(bI[0:127, 0:1], take[0:127, 0:1], src_i[0:127, C - 1:C])
                    nc.vector.copy_predicated(src_i[0:127, C - 1:C], take[0:127, 0:1], bI[0:127, 1:2])
                    nc.sync.dma_start(src_i[1:128, 0:1], bI[0:127, 0:1])

                pid = sortpool.tile([128, C], dt.int32, tag=f"pid{b}")
                nc.vector.tensor_copy(pid[:], src_i[:])
                nc.sync.dma_start(pidx_o[b, :], pid[:])

    nc.compile()
    return nc


_CACHE = {}
LAST_RESULTS = None


def _get_program(NB, N, c_norm, bp_val, enable_sort=True):
    key = (NB, N, float(c_norm), float(bp_val), enable_sort)
    if key not in _CACHE:
        _CACHE[key] = build_program(NB, N, c_norm, bp_val, enable_sort)
    return _CACHE[key]


def kernel(xyz, f, W1, b1, W2, b2, W3, b3, Wp, bp,
           Wd1, bd1, Wd2, bd2, Wd3, bd3):
    B, N = xyz.shape[0], xyz.shape[1]
    n_out = int(N * PERCENT)
    c_norm = np.linalg.norm(Wp.astype(np.float32))
    nc = _get_program(B // N_CORES, N, c_norm, float(bp[0]))

    w_common = dict(W1=np.ascontiguousarray(W1, np.float32), b1=b1.astype(np.float32),
                    W2=np.ascontiguousarray(W2, np.float32), b2=b2.astype(np.float32),
                    W3=np.ascontiguousarray(W3, np.float32), b3=b3.astype(np.float32),
                    Wp=np.ascontiguousarray(Wp, np.float32),
                    Wd1=np.ascontiguousarray(Wd1, np.float32), bd1=bd1.astype(np.float32),
                    Wd2=np.ascontiguousarray(Wd2, np.float32), bd2=bd2.astype(np.float32),
                    Wd3=np.ascontiguousarray(Wd3, np.float32), bd3=bd3.astype(np.float32))
    in_maps = []
    nb = B // N_CORES
    for k in range(N_CORES):
        sl = slice(k * nb, (k + 1) * nb)
        in_maps.append(dict(
            fT=np.ascontiguousarray(np.transpose(f[sl], (0, 2, 1)), np.float32),
            xyzT=np.ascontiguousarray(np.transpose(xyz[sl], (0, 2, 1)), np.float32),
            **w_common))

    trace = bool(int(os.environ.get("BASS_KERNEL_TRACE", "0")))
    res = run_bass_kernel_spmd(nc, in_maps, list(range(N_CORES)), trace=trace)
    global LAST_RESULTS
    LAST_RESULTS = res

    prob_idx = np.concatenate([r["pidx_o"] for r in res.results], axis=0)
    est_full = np.concatenate(
        [np.transpose(r["est_o"], (0, 2, 1)) for r in res.results], axis=0)
    clean_idx = prob_idx[:, n_out:]
    est_xyz = np.take_along_axis(est_full, clean_idx[..., None], axis=1)
    return prob_idx.astype(np.int32), est_xyz.astype(np.float32)
